# revision 11
# baseline (speedup 1.0000x reference)
"""TRN2 Bass kernel for nn_COV_75359496176097.

reference():
    B2 = B[0]                               # (8192, 8192)
    rn = sqrt(1 / sum(B2*B2, axis=1))       # row norms
    A  = rn * B2 * exp(tile(logstd, 64))[:, None]
    samples = tile(mu,64) + einsum('mk,bk->bm', A, eps[:,:,0])
    returns (mu_out, logvar, samples), each (128, 64, 128)

Strategy: shard B by rows across 8 cores (1024 rows each, no collectives).
Each core computes out[b, r] = sum_k eps[k, b] * B[r, k] on the PE
(eps k-tile stationary, B^T k-tile moving, PSUM-accumulated over 64
k-tiles) plus row norms via on-the-fly squares (DVE, bf16) reduced with
an all-ones stationary matmul (which also broadcasts the per-row sums
across all 128 output partitions for free). Epilogue applies
out = acc * sqrt(1/sumsq) * exp(logstd_rep) + mu_rep and DMAs out.

Raw Bass (not Tile): hardware allows at most ONE semaphore wait per
instruction, and this dataflow (each DMA'd tile consumed by both PE and
DVE) needs transitive cross-engine reasoning Tile doesn't do. Manual
scheme: per-slot DMA-completion semaphores; PE's norm matmul for tile t
waits on DVE's square, so "PE finished tile t" implies every consumer of
slot t is done; the DMA issuer throttles on that single PE semaphore.

Each k-tile's B^T slice and eps^T slice are packed side by side in one
host-prepared tensor so a k-tile needs exactly one DMA.
"""

import sys
from contextlib import ExitStack

if "/opt/trn_rl_repo" not in sys.path:
    sys.path.insert(0, "/opt/trn_rl_repo")

import numpy as np
import ml_dtypes

import concourse.bacc as bacc
import concourse.mybir as mybir
from concourse import bass_utils

Z = 128
NS = 64
M = Z * NS          # 8192
BATCH = 128
NCORES = 8
RPC = M // NCORES   # 1024 rows of B per core
KT = M // 128       # 64 k-tiles
W = RPC + BATCH     # 1152 packed row width
NB = 8              # B-tile SBUF slots (DMA prefetch depth)
G = 4               # k-tiles per PE matmul group (LDW pipelining)

f32 = mybir.dt.float32
f32r = mybir.dt.float32r
bf16 = mybir.dt.bfloat16

# "f32r": fp32r matmul pair per k-tile (fast, ~fp32-ish precision - measured)
# "split": bf16 hi/lo 3-matmul decomposition (precision fallback)
MODE = "f32r"

_nc_cache = {}


def _build(mode):
    nc = bacc.Bacc("TRN2", debug=False)

    if mode == "f32r":
        bte_d = nc.dram_tensor("bte", (M, W), f32r, kind="ExternalInput")
    else:
        bh_d = nc.dram_tensor("bteh", (M, W), bf16, kind="ExternalInput")
        bl_d = nc.dram_tensor("btel", (M, W), bf16, kind="ExternalInput")
    els_d = nc.dram_tensor("els", (BATCH, RPC), f32, kind="ExternalInput")
    mu_d = nc.dram_tensor("mu", (BATCH, RPC), f32, kind="ExternalInput")
    out_d = nc.dram_tensor("out", (BATCH, RPC), f32, kind="ExternalOutput")

    with ExitStack() as ctx:
        e = ctx.enter_context
        if mode == "f32r":
            slots = [e(nc.sbuf_tensor(f"slot{i}", [128, W], f32r)) for i in range(NB)]
        else:
            slots = [e(nc.sbuf_tensor(f"slot{i}", [128, W], bf16)) for i in range(NB)]
            slots_l = [e(nc.sbuf_tensor(f"slotl{i}", [128, W], bf16)) for i in range(NB)]
            s_dma_l = [e(nc.semaphore(name=f"s_dma_l{i}")) for i in range(NB)]
        sq = [e(nc.sbuf_tensor(f"sq{i}", [128, RPC], bf16)) for i in range(NB)]
        ones = e(nc.sbuf_tensor("ones", [128, 128], bf16))
        els_sb = e(nc.sbuf_tensor("els_sb", [128, RPC], f32))
        mu_sb = e(nc.sbuf_tensor("mu_sb", [128, RPC], f32))
        inv_sb = e(nc.sbuf_tensor("inv_sb", [128, RPC], f32))
        rn_sb = e(nc.sbuf_tensor("rn_sb", [128, RPC], f32))
        scale_sb = e(nc.sbuf_tensor("scale_sb", [128, RPC], f32))
        out_sb = e(nc.sbuf_tensor("out_sb", [128, RPC], f32))
        acc = e(nc.psum_tensor([128, RPC], f32))
        nrm = e(nc.psum_tensor([128, RPC], f32))

        s_dma = [e(nc.semaphore(name=f"s_dma{i}")) for i in range(NB)]
        s_cst = e(nc.semaphore())
        s_pe = e(nc.semaphore())
        s_dve = e(nc.semaphore())
        s_x = e(nc.semaphore())
        s_out = e(nc.semaphore())
        s_od = e(nc.semaphore())

        block = e(nc.Block())

        @block.sync
        def _(sync):
            for t in range(KT):
                sl = slice(t * 128, (t + 1) * 128)
                if t == NB:
                    # constants only needed by the epilogue; issue after the
                    # first wave of B-tile DMAs so the PE starts sooner
                    sync.dma_start(els_sb[:], els_d.ap()[:, :]).then_inc(
                        s_cst, 16
                    )
                    sync.dma_start(mu_sb[:], mu_d.ap()[:, :]).then_inc(
                        s_cst, 16
                    )
                if t >= NB:
                    # slot free once PE's norm matmul group of tile t-NB
                    # retired (transitively implies DVE's square is done too)
                    sync.wait_ge(s_pe, (t - NB) // G + 1)
                if mode == "f32r":
                    sync.dma_start(
                        slots[t % NB][:], bte_d.ap()[sl, :]
                    ).then_inc(s_dma[t % NB], 16)
                else:
                    sync.dma_start(
                        slots[t % NB][:], bh_d.ap()[sl, :]
                    ).then_inc(s_dma[t % NB], 16)
                    sync.dma_start(
                        slots_l[t % NB][:], bl_d.ap()[sl, :]
                    ).then_inc(s_dma_l[t % NB], 16)
            sync.wait_ge(s_out, 1)
            sync.dma_start(out_d.ap()[:, :], out_sb[:]).then_inc(s_od, 16)
            sync.wait_ge(s_od, 16)
            sync.nop()

        @block.tensor
        def _(tensor):
            # k-tiles processed in groups of G so same-dtype matmuls run
            # back-to-back (weight loads pipeline; fp32r MMs poison FWL)
            for g in range(KT // G):
                tiles = range(g * G, (g + 1) * G)
                for t in tiles:
                    st, sp = t == 0, t == KT - 1
                    s = t % NB
                    need = 16 * (t // NB + 1)
                    tensor.wait_ge(s_dma[s], need)
                    if mode == "f32r":
                        eps_v = slots[s][:, RPC:W]
                        for h in range(RPC // 512):
                            hs = slice(h * 512, (h + 1) * 512)
                            nc.tensor.matmul(
                                acc[:, hs], eps_v, slots[s][:, hs],
                                start=st, stop=sp,
                            )
                    else:
                        eh_v = slots[s][:, RPC:W]
                        el_v = slots_l[s][:, RPC:W]
                        for h in range(RPC // 512):
                            hs = slice(h * 512, (h + 1) * 512)
                            nc.tensor.matmul(
                                acc[:, hs], eh_v, slots[s][:, hs],
                                start=st, stop=False,
                            )
                        tensor.wait_ge(s_dma_l[s], need)
                        for h in range(RPC // 512):
                            hs = slice(h * 512, (h + 1) * 512)
                            nc.tensor.matmul(
                                acc[:, hs], el_v, slots[s][:, hs],
                                start=False, stop=False,
                            )
                            nc.tensor.matmul(
                                acc[:, hs], eh_v, slots_l[s][:, hs],
                                start=False, stop=sp,
                            )
                tensor.wait_ge(s_dve, (g + 1) * G)
                for t in tiles:
                    st, sp = t == 0, t == KT - 1
                    s = t % NB
                    for h in range(RPC // 512):
                        hs = slice(h * 512, (h + 1) * 512)
                        ins = nc.tensor.matmul(
                            nrm[:, hs], ones[:], sq[s][:, hs], start=st, stop=sp
                        )
                ins.then_inc(s_pe, 1)

        @block.vector
        def _(vector):
            nc.vector.memset(ones[:], 1.0)
            for t in range(KT):
                s = t % NB
                # the slot DMA only fired after PE retired tile t-NB, so the
                # sq[s] anti-dependency (PE read of square t-NB) is implied
                vector.wait_ge(s_dma[s], 16 * (t // NB + 1))
                if mode == "f32r":
                    btf = slots[s][:, 0:RPC].bitcast(f32)
                else:
                    btf = slots[s][:, 0:RPC]
                ins = nc.vector.tensor_mul(sq[s][:], btf, btf)
                ins.then_inc(s_dve, 1)
            # epilogue: out = acc * sqrt(1/nrm) * els + mu
            vector.wait_ge(s_pe, KT // G)
            nc.vector.reciprocal(inv_sb[:], nrm[:]).then_inc(s_x, 1)
            vector.wait_ge(s_cst, 32)
            vector.drain()
            vector.wait_ge(s_x, 2)
            nc.vector.tensor_mul(scale_sb[:], rn_sb[:], els_sb[:])
            vector.drain()
            nc.vector.tensor_mul(out_sb[:], acc[:], scale_sb[:])
            vector.drain()
            nc.vector.tensor_add(out_sb[:], out_sb[:], mu_sb[:]).then_inc(
                s_out, 1
            )

        @block.scalar
        def _(scalar):
            scalar.wait_ge(s_x, 1)
            nc.scalar.sqrt(rn_sb[:], inv_sb[:]).then_inc(s_x, 1)

    nc.compile()
    return nc


def _get_nc(mode):
    if mode not in _nc_cache:
        _nc_cache[mode] = _build(mode)
    return _nc_cache[mode]


def _split_bf16(x):
    hi = x.astype(ml_dtypes.bfloat16)
    lo = (x - hi.astype(np.float32)).astype(ml_dtypes.bfloat16)
    return hi, lo


def _prep_inputs(mu, logstd, B, eps):
    B2 = B[0]
    epst = np.ascontiguousarray(eps[:, :, 0].T)        # (M, BATCH)
    mu_rep = np.tile(mu[0], NS)                        # (M,)
    logstd_rep = np.tile(logstd, NS)                   # (M,)
    els_rep = np.exp(logstd_rep).astype(np.float32)    # (M,)

    in_maps = []
    for c in range(NCORES):
        rows = slice(c * RPC, (c + 1) * RPC)
        bte = np.empty((M, W), dtype=np.float32)
        bte[:, 0:RPC] = B2[rows, :].T
        bte[:, RPC:W] = epst
        m = {
            "els": np.ascontiguousarray(
                np.broadcast_to(els_rep[rows][None, :], (BATCH, RPC))
            ),
            "mu": np.ascontiguousarray(
                np.broadcast_to(mu_rep[rows][None, :], (BATCH, RPC))
            ),
        }
        if MODE == "f32r":
            m["bte"] = bte
        else:
            m["bteh"], m["btel"] = _split_bf16(bte)
        in_maps.append(m)
    return in_maps, mu_rep, logstd_rep


def _run(mu, logstd, B, eps, batch_size, trace=False, trace_kwargs=None):
    mu = np.asarray(mu, dtype=np.float32)
    logstd = np.asarray(logstd, dtype=np.float32)
    B = np.asarray(B, dtype=np.float32)
    eps = np.asarray(eps, dtype=np.float32)
    b = int(batch_size)
    assert B.shape == (1, M, M) and eps.shape == (b, M, 1) and b == BATCH

    in_maps, mu_rep, logstd_rep = _prep_inputs(mu, logstd, B, eps)

    nc = _get_nc(MODE)
    kw = {}
    if trace:
        kw = dict(trace=True, trace_cores=list(range(NCORES)))
        if trace_kwargs:
            kw.update(trace_kwargs)
    res = bass_utils.run_bass_kernel_spmd(
        nc, in_maps, core_ids=list(range(NCORES)), **kw
    )

    samples_bm = np.concatenate(
        [res.results[c]["out"] for c in range(NCORES)], axis=1
    )  # (BATCH, M)
    samples = samples_bm.reshape(b, NS, Z)
    mu_out = np.broadcast_to(mu_rep[None, :], (b, M)).reshape(b, NS, Z).copy()
    logvar = (
        np.broadcast_to(2.0 * logstd_rep[None, :], (b, M)).reshape(b, NS, Z).copy()
    )
    return (mu_out, logvar, samples), res


def kernel(mu, logstd, B, eps, batch_size):
    outs, _ = _run(mu, logstd, B, eps, batch_size, trace=False)
    return outs


# revision 12
# speedup vs baseline: 1.0765x; 1.0765x over previous
"""TRN2 Bass kernel for nn_COV_75359496176097.

reference():
    B2 = B[0]                               # (8192, 8192)
    rn = sqrt(1 / sum(B2*B2, axis=1))       # row norms
    A  = rn * B2 * exp(tile(logstd, 64))[:, None]
    samples = tile(mu,64) + einsum('mk,bk->bm', A, eps[:,:,0])
    returns (mu_out, logvar, samples), each (128, 64, 128)

Strategy: shard B by rows across 8 cores (1024 rows each, no collectives).
Each core computes out[b, r] = sum_k eps[k, b] * B[r, k] on the PE
(eps k-tile stationary, B^T k-tile moving, PSUM-accumulated over 64
k-tiles) plus row norms via on-the-fly squares (DVE, bf16) reduced with
an all-ones stationary matmul (which also broadcasts the per-row sums
across all 128 output partitions for free). Epilogue applies
out = acc * sqrt(1/sumsq) * exp(logstd_rep) + mu_rep and DMAs out.

Raw Bass (not Tile): hardware allows at most ONE semaphore wait per
instruction, and this dataflow (each DMA'd tile consumed by both PE and
DVE) needs transitive cross-engine reasoning Tile doesn't do. Manual
scheme: per-slot DMA-completion semaphores; PE's norm matmul for tile t
waits on DVE's square, so "PE finished tile t" implies every consumer of
slot t is done; the DMA issuer throttles on that single PE semaphore.

Each k-tile's B^T slice and eps^T slice are packed side by side in one
host-prepared tensor so a k-tile needs exactly one DMA.
"""

import sys
from contextlib import ExitStack

if "/opt/trn_rl_repo" not in sys.path:
    sys.path.insert(0, "/opt/trn_rl_repo")

import numpy as np
import ml_dtypes

import concourse.bacc as bacc
import concourse.mybir as mybir
from concourse import bass_utils

Z = 128
NS = 64
M = Z * NS          # 8192
BATCH = 128
NCORES = 8
RPC = M // NCORES   # 1024 rows of B per core
KT = M // 128       # 64 k-tiles
W = RPC + BATCH     # 1152 packed row width
NB = 8              # B-tile SBUF slots (DMA prefetch depth)
G = 1               # k-tiles per PE matmul group

f32 = mybir.dt.float32
f32r = mybir.dt.float32r
bf16 = mybir.dt.bfloat16

# "f32r": fp32r matmul pair per k-tile (fast, ~fp32-ish precision - measured)
# "split": bf16 hi/lo 3-matmul decomposition (precision fallback)
MODE = "f32r"

_nc_cache = {}


def _build(mode):
    nc = bacc.Bacc("TRN2", debug=False)

    if mode == "f32r":
        bte_d = nc.dram_tensor("bte", (M, W), f32r, kind="ExternalInput")
    else:
        bh_d = nc.dram_tensor("bteh", (M, W), bf16, kind="ExternalInput")
        bl_d = nc.dram_tensor("btel", (M, W), bf16, kind="ExternalInput")
    els_d = nc.dram_tensor("els", (BATCH, RPC), f32, kind="ExternalInput")
    mu_d = nc.dram_tensor("mu", (BATCH, RPC), f32, kind="ExternalInput")
    out_d = nc.dram_tensor("out", (BATCH, RPC), f32, kind="ExternalOutput")

    with ExitStack() as ctx:
        e = ctx.enter_context
        if mode == "f32r":
            slots = [e(nc.sbuf_tensor(f"slot{i}", [128, W], f32r)) for i in range(NB)]
        else:
            slots = [e(nc.sbuf_tensor(f"slot{i}", [128, W], bf16)) for i in range(NB)]
            slots_l = [e(nc.sbuf_tensor(f"slotl{i}", [128, W], bf16)) for i in range(NB)]
            s_dma_l = [e(nc.semaphore(name=f"s_dma_l{i}")) for i in range(NB)]
        sq = [e(nc.sbuf_tensor(f"sq{i}", [128, RPC], bf16)) for i in range(NB)]
        ones = e(nc.sbuf_tensor("ones", [128, 128], bf16))
        els_sb = e(nc.sbuf_tensor("els_sb", [128, RPC], f32))
        mu_sb = e(nc.sbuf_tensor("mu_sb", [128, RPC], f32))
        inv_sb = e(nc.sbuf_tensor("inv_sb", [128, RPC], f32))
        rn_sb = e(nc.sbuf_tensor("rn_sb", [128, RPC], f32))
        scale_sb = e(nc.sbuf_tensor("scale_sb", [128, RPC], f32))
        out_sb = e(nc.sbuf_tensor("out_sb", [128, RPC], f32))
        acc = e(nc.psum_tensor([128, RPC], f32))
        nrm = e(nc.psum_tensor([128, RPC], f32))

        s_dma = [e(nc.semaphore(name=f"s_dma{i}")) for i in range(NB)]
        s_cst = e(nc.semaphore())
        s_pe = e(nc.semaphore())
        s_dve = e(nc.semaphore())
        s_act = e(nc.semaphore())
        s_x = e(nc.semaphore())
        s_out = e(nc.semaphore())
        s_od = e(nc.semaphore())

        block = e(nc.Block())

        @block.sync
        def _(sync):
            for t in range(KT):
                sl = slice(t * 128, (t + 1) * 128)
                if t == NB:
                    # constants only needed by the epilogue; issue after the
                    # first wave of B-tile DMAs so the PE starts sooner
                    sync.dma_start(els_sb[:], els_d.ap()[:, :]).then_inc(
                        s_cst, 16
                    )
                    sync.dma_start(mu_sb[:], mu_d.ap()[:, :]).then_inc(
                        s_cst, 16
                    )
                if t >= NB:
                    # slot free once PE's norm matmul group of tile t-NB
                    # retired (transitively implies DVE's square is done too)
                    sync.wait_ge(s_pe, (t - NB) // G + 1)
                if mode == "f32r":
                    sync.dma_start(
                        slots[t % NB][:], bte_d.ap()[sl, :]
                    ).then_inc(s_dma[t % NB], 16)
                else:
                    sync.dma_start(
                        slots[t % NB][:], bh_d.ap()[sl, :]
                    ).then_inc(s_dma[t % NB], 16)
                    sync.dma_start(
                        slots_l[t % NB][:], bl_d.ap()[sl, :]
                    ).then_inc(s_dma_l[t % NB], 16)
            sync.wait_ge(s_out, 1)
            sync.dma_start(out_d.ap()[:, :], out_sb[:]).then_inc(s_od, 16)
            sync.wait_ge(s_od, 16)
            sync.nop()

        @block.tensor
        def _(tensor):
            # k-tiles processed in groups of G so same-dtype matmuls run
            # back-to-back (weight loads pipeline; fp32r MMs poison FWL)
            for g in range(KT // G):
                tiles = range(g * G, (g + 1) * G)
                for t in tiles:
                    st, sp = t == 0, t == KT - 1
                    s = t % NB
                    need = 16 * (t // NB + 1)
                    tensor.wait_ge(s_dma[s], need)
                    if mode == "f32r":
                        eps_v = slots[s][:, RPC:W]
                        for h in range(RPC // 512):
                            hs = slice(h * 512, (h + 1) * 512)
                            nc.tensor.matmul(
                                acc[:, hs], eps_v, slots[s][:, hs],
                                start=st, stop=sp,
                            )
                    else:
                        eh_v = slots[s][:, RPC:W]
                        el_v = slots_l[s][:, RPC:W]
                        for h in range(RPC // 512):
                            hs = slice(h * 512, (h + 1) * 512)
                            nc.tensor.matmul(
                                acc[:, hs], eh_v, slots[s][:, hs],
                                start=st, stop=False,
                            )
                        tensor.wait_ge(s_dma_l[s], need)
                        for h in range(RPC // 512):
                            hs = slice(h * 512, (h + 1) * 512)
                            nc.tensor.matmul(
                                acc[:, hs], el_v, slots[s][:, hs],
                                start=False, stop=False,
                            )
                            nc.tensor.matmul(
                                acc[:, hs], eh_v, slots_l[s][:, hs],
                                start=False, stop=sp,
                            )
                tensor.wait_ge(s_act, (g + 1) * G)
                for t in tiles:
                    st, sp = t == 0, t == KT - 1
                    s = t % NB
                    for h in range(RPC // 512):
                        hs = slice(h * 512, (h + 1) * 512)
                        ins = nc.tensor.matmul(
                            nrm[:, hs], ones[:], sq[s][:, hs], start=st, stop=sp
                        )
                ins.then_inc(s_pe, 1)

        @block.vector
        def _(vector):
            nc.vector.memset(ones[:], 1.0).then_inc(s_dve, 1)
            # epilogue: out = acc * sqrt(1/nrm) * els + mu
            vector.wait_ge(s_pe, KT // G)
            nc.vector.reciprocal(inv_sb[:], nrm[:]).then_inc(s_x, 1)
            vector.wait_ge(s_cst, 32)
            vector.drain()
            vector.wait_ge(s_x, 2)
            nc.vector.tensor_mul(scale_sb[:], rn_sb[:], els_sb[:])
            vector.drain()
            nc.vector.tensor_mul(out_sb[:], acc[:], scale_sb[:])
            vector.drain()
            nc.vector.tensor_add(out_sb[:], out_sb[:], mu_sb[:]).then_inc(
                s_out, 1
            )

        @block.scalar
        def _(scalar):
            # squares on the otherwise-idle ACT engine; first one also gates
            # on the DVE memset of `ones` so s_act transitively covers it
            scalar.wait_ge(s_dve, 1)
            for t in range(KT):
                s = t % NB
                # the slot DMA only fired after PE retired tile t-NB, so the
                # sq[s] anti-dependency (PE read of square t-NB) is implied
                scalar.wait_ge(s_dma[s], 16 * (t // NB + 1))
                if mode == "f32r":
                    btf = slots[s][:, 0:RPC].bitcast(f32)
                else:
                    btf = slots[s][:, 0:RPC]
                nc.scalar.square(sq[s][:], btf).then_inc(s_act, 1)
            scalar.wait_ge(s_x, 1)
            nc.scalar.sqrt(rn_sb[:], inv_sb[:]).then_inc(s_x, 1)

    nc.compile()
    return nc


def _get_nc(mode):
    if mode not in _nc_cache:
        _nc_cache[mode] = _build(mode)
    return _nc_cache[mode]


def _split_bf16(x):
    hi = x.astype(ml_dtypes.bfloat16)
    lo = (x - hi.astype(np.float32)).astype(ml_dtypes.bfloat16)
    return hi, lo


def _prep_inputs(mu, logstd, B, eps):
    B2 = B[0]
    epst = np.ascontiguousarray(eps[:, :, 0].T)        # (M, BATCH)
    mu_rep = np.tile(mu[0], NS)                        # (M,)
    logstd_rep = np.tile(logstd, NS)                   # (M,)
    els_rep = np.exp(logstd_rep).astype(np.float32)    # (M,)

    in_maps = []
    for c in range(NCORES):
        rows = slice(c * RPC, (c + 1) * RPC)
        bte = np.empty((M, W), dtype=np.float32)
        bte[:, 0:RPC] = B2[rows, :].T
        bte[:, RPC:W] = epst
        m = {
            "els": np.ascontiguousarray(
                np.broadcast_to(els_rep[rows][None, :], (BATCH, RPC))
            ),
            "mu": np.ascontiguousarray(
                np.broadcast_to(mu_rep[rows][None, :], (BATCH, RPC))
            ),
        }
        if MODE == "f32r":
            m["bte"] = bte
        else:
            m["bteh"], m["btel"] = _split_bf16(bte)
        in_maps.append(m)
    return in_maps, mu_rep, logstd_rep


def _run(mu, logstd, B, eps, batch_size, trace=False, trace_kwargs=None):
    mu = np.asarray(mu, dtype=np.float32)
    logstd = np.asarray(logstd, dtype=np.float32)
    B = np.asarray(B, dtype=np.float32)
    eps = np.asarray(eps, dtype=np.float32)
    b = int(batch_size)
    assert B.shape == (1, M, M) and eps.shape == (b, M, 1) and b == BATCH

    in_maps, mu_rep, logstd_rep = _prep_inputs(mu, logstd, B, eps)

    nc = _get_nc(MODE)
    kw = {}
    if trace:
        kw = dict(trace=True, trace_cores=list(range(NCORES)))
        if trace_kwargs:
            kw.update(trace_kwargs)
    res = bass_utils.run_bass_kernel_spmd(
        nc, in_maps, core_ids=list(range(NCORES)), **kw
    )

    samples_bm = np.concatenate(
        [res.results[c]["out"] for c in range(NCORES)], axis=1
    )  # (BATCH, M)
    samples = samples_bm.reshape(b, NS, Z)
    mu_out = np.broadcast_to(mu_rep[None, :], (b, M)).reshape(b, NS, Z).copy()
    logvar = (
        np.broadcast_to(2.0 * logstd_rep[None, :], (b, M)).reshape(b, NS, Z).copy()
    )
    return (mu_out, logvar, samples), res


def kernel(mu, logstd, B, eps, batch_size):
    outs, _ = _run(mu, logstd, B, eps, batch_size, trace=False)
    return outs


# revision 15
# speedup vs baseline: 1.0986x; 1.0205x over previous
"""TRN2 Bass kernel for nn_COV_75359496176097.

reference():
    B2 = B[0]                               # (8192, 8192)
    rn = sqrt(1 / sum(B2*B2, axis=1))       # row norms
    A  = rn * B2 * exp(tile(logstd, 64))[:, None]
    samples = tile(mu,64) + einsum('mk,bk->bm', A, eps[:,:,0])
    returns (mu_out, logvar, samples), each (128, 64, 128)

Strategy: shard B by rows across 8 cores (1024 rows each, no collectives).
Each core computes out[b, r] = sum_k eps[k, b] * B[r, k] on the PE
(eps k-tile stationary fp32r, B^T k-tile moving fp32r, PSUM-accumulated
over 64 k-tiles). Row norms: ACT squares each B^T tile (fp32), DVE
accumulates the squares elementwise across k-tiles (ping-pong pair of
accumulators to avoid back-to-back pipeline hazards), and a single pair
of all-ones fp32 matmuls in the tail does the 128-partition reduction —
broadcast across all output partitions for free. Epilogue applies
out = acc * sqrt(1/sumsq) * exp(logstd_rep) + mu_rep and DMAs out.

Raw Bass (not Tile): hardware allows at most ONE semaphore wait per
instruction, and this dataflow (each DMA'd tile consumed by PE and ACT)
needs transitive cross-engine reasoning Tile doesn't do. Manual scheme:
per-slot DMA-completion semaphores; DVE's accumulate for tile t waits on
both PE (via a nop) and ACT, so "DVE retired tile t" implies every
consumer of slot t is done; the DMA issuer throttles on that single DVE
semaphore.

Each k-tile's B^T slice and eps^T slice are packed side by side in one
host-prepared tensor so a k-tile needs exactly one DMA.
"""

import sys
from contextlib import ExitStack

if "/opt/trn_rl_repo" not in sys.path:
    sys.path.insert(0, "/opt/trn_rl_repo")

import numpy as np

import concourse.bacc as bacc
import concourse.mybir as mybir
from concourse import bass_utils
from concourse.dve_ops import RECIPROCAL_APPROX_NR

Z = 128
NS = 64
M = Z * NS          # 8192
BATCH = 128
NCORES = 8
RPC = M // NCORES   # 1024 rows of B per core
KT = M // 128       # 64 k-tiles
W = RPC + BATCH     # 1152 packed row width
NB = 8              # B-tile SBUF slots (DMA prefetch depth)

f32 = mybir.dt.float32
f32r = mybir.dt.float32r

_nc_cache = {}


def _build():
    nc = bacc.Bacc("TRN2", debug=False)

    bte_d = nc.dram_tensor("bte", (M, W), f32r, kind="ExternalInput")
    els_d = nc.dram_tensor("els", (BATCH, RPC), f32, kind="ExternalInput")
    mu_d = nc.dram_tensor("mu", (BATCH, RPC), f32, kind="ExternalInput")
    out_d = nc.dram_tensor("out", (BATCH, RPC), f32, kind="ExternalOutput")

    with ExitStack() as ctx:
        e = ctx.enter_context
        slots = [e(nc.sbuf_tensor(f"slot{i}", [128, W], f32r)) for i in range(NB)]
        sq = [e(nc.sbuf_tensor(f"sq{i}", [128, RPC], f32)) for i in range(NB)]
        sqa0 = e(nc.sbuf_tensor("sqa0", [128, RPC], f32))
        sqa1 = e(nc.sbuf_tensor("sqa1", [128, RPC], f32))
        ones = e(nc.sbuf_tensor("ones", [128, 128], f32))
        els_sb = e(nc.sbuf_tensor("els_sb", [128, RPC], f32))
        mu_sb = e(nc.sbuf_tensor("mu_sb", [128, RPC], f32))
        inv_sb = e(nc.sbuf_tensor("inv_sb", [128, RPC], f32))
        rn_sb = e(nc.sbuf_tensor("rn_sb", [128, RPC], f32))
        scale_sb = e(nc.sbuf_tensor("scale_sb", [128, RPC], f32))
        out_sb = e(nc.sbuf_tensor("out_sb", [128, RPC], f32))
        acc = e(nc.psum_tensor([128, RPC], f32))
        nrm = e(nc.psum_tensor([128, RPC], f32))

        s_dma = [e(nc.semaphore(name=f"s_dma{i}")) for i in range(NB)]
        s_cst = e(nc.semaphore(name="s_cst"))
        s_pe = e(nc.semaphore(name="s_pe"))
        s_dve = e(nc.semaphore(name="s_dve"))
        s_act = e(nc.semaphore(name="s_act"))
        s_x = e(nc.semaphore(name="s_x"))
        s_out = e(nc.semaphore(name="s_out"))
        s_od = e(nc.semaphore(name="s_od"))

        block = e(nc.Block())

        @block.sync
        def _(sync):
            for t in range(KT):
                sl = slice(t * 128, (t + 1) * 128)
                if t == NB:
                    # constants only needed by the epilogue; issue after the
                    # first wave of B-tile DMAs so the PE starts sooner
                    sync.dma_start(els_sb[:], els_d.ap()[:, :]).then_inc(
                        s_cst, 16
                    )
                    sync.dma_start(mu_sb[:], mu_d.ap()[:, :]).then_inc(
                        s_cst, 16
                    )
                if t >= NB:
                    # slot free once DVE's accumulate of tile t-NB retired
                    # (transitively implies PE's matmuls and ACT's square)
                    sync.wait_ge(s_dve, t - NB + 1)
                sync.dma_start(
                    slots[t % NB][:], bte_d.ap()[sl, :]
                ).then_inc(s_dma[t % NB], 16)
            sync.wait_ge(s_out, 1)
            sync.dma_start(out_d.ap()[:, :], out_sb[:]).then_inc(s_od, 16)
            sync.wait_ge(s_od, 16)
            sync.nop()

        @block.tensor
        def _(tensor):
            for t in range(KT):
                st, sp = t == 0, t == KT - 1
                s = t % NB
                tensor.wait_ge(s_dma[s], 16 * (t // NB + 1))
                eps_v = slots[s][:, RPC:W]
                for h in range(RPC // 512):
                    hs = slice(h * 512, (h + 1) * 512)
                    ins = nc.tensor.matmul(
                        acc[:, hs], eps_v, slots[s][:, hs], start=st, stop=sp
                    )
                ins.then_inc(s_pe, 1)
            # tail: partition-reduce the square accumulator (plain fp32
            # matmul against all-ones — broadcasts the sums to all 128
            # output partitions as a side effect)
            tensor.wait_ge(s_x, 1)
            for h in range(RPC // 512):
                hs = slice(h * 512, (h + 1) * 512)
                ins = nc.tensor.matmul(
                    nrm[:, hs], ones[:], sqa0[:, hs], start=True, stop=True
                )
            ins.then_inc(s_x, 1)

        @block.scalar
        def _(scalar):
            for t in range(KT):
                s = t % NB
                # the slot DMA only fired after DVE retired tile t-NB, so
                # the sq[s] anti-dependency (DVE read of square t-NB) holds
                scalar.wait_ge(s_dma[s], 16 * (t // NB + 1))
                nc.scalar.square(
                    sq[s][:], slots[s][:, 0:RPC].bitcast(f32)
                ).then_inc(s_act, 1)
            scalar.wait_ge(s_x, 3)
            nc.scalar.sqrt(rn_sb[:], inv_sb[:]).then_inc(s_x, 1)

        @block.vector
        def _(vector):
            nc.vector.memset(ones[:], 1.0)
            nc.vector.memset(sqa0[:], 0.0)
            nc.vector.memset(sqa1[:], 0.0)
            vector.drain()
            for t in range(KT):
                s = t % NB
                sqa = sqa0 if t % 2 == 0 else sqa1
                vector.wait_ge(s_pe, t + 1)
                vector.nop()
                if t >= 2:
                    # self-wait orders this add after the t-2 add on the same
                    # ping-pong accumulator (already retired; no stall)
                    vector.wait_ge(s_dve, t - 1)
                    vector.nop()
                vector.wait_ge(s_act, t + 1)
                ins = nc.vector.tensor_add(sqa[:], sqa[:], sq[s][:])
                ins.then_inc(s_dve, 1)
            # merge ping-pong accumulators (drain: adjacent RAW on sqa1)
            vector.drain()
            nc.vector.tensor_add(sqa0[:], sqa0[:], sqa1[:]).then_inc(s_x, 1)
            # nrm ready at s_x>=2; reciprocal at ~2 ULP then sqrt on ACT
            vector.wait_ge(s_x, 2)
            nc.vector.reciprocal_approx_fast(out=scale_sb[:], in_=nrm[:])
            vector.drain()
            nc.vector._custom_dve(
                RECIPROCAL_APPROX_NR,
                out=inv_sb[:],
                in0=nrm[:],
                in1=scale_sb[:],
                s0=2.0,
            ).then_inc(s_x, 1)
            vector.wait_ge(s_cst, 32)
            vector.drain()
            vector.wait_ge(s_x, 4)
            nc.vector.tensor_mul(scale_sb[:], rn_sb[:], els_sb[:])
            vector.drain()
            nc.vector.tensor_mul(out_sb[:], acc[:], scale_sb[:])
            vector.drain()
            nc.vector.tensor_add(out_sb[:], out_sb[:], mu_sb[:]).then_inc(
                s_out, 1
            )

    nc.compile()
    return nc


def _get_nc():
    if "nc" not in _nc_cache:
        _nc_cache["nc"] = _build()
    return _nc_cache["nc"]


def _prep_inputs(mu, logstd, B, eps):
    B2 = B[0]
    epst = np.ascontiguousarray(eps[:, :, 0].T)        # (M, BATCH)
    mu_rep = np.tile(mu[0], NS)                        # (M,)
    logstd_rep = np.tile(logstd, NS)                   # (M,)
    els_rep = np.exp(logstd_rep).astype(np.float32)    # (M,)

    in_maps = []
    for c in range(NCORES):
        rows = slice(c * RPC, (c + 1) * RPC)
        bte = np.empty((M, W), dtype=np.float32)
        bte[:, 0:RPC] = B2[rows, :].T
        bte[:, RPC:W] = epst
        in_maps.append(
            {
                "bte": bte,
                "els": np.ascontiguousarray(
                    np.broadcast_to(els_rep[rows][None, :], (BATCH, RPC))
                ),
                "mu": np.ascontiguousarray(
                    np.broadcast_to(mu_rep[rows][None, :], (BATCH, RPC))
                ),
            }
        )
    return in_maps, mu_rep, logstd_rep


def _run(mu, logstd, B, eps, batch_size, trace=False, trace_kwargs=None):
    mu = np.asarray(mu, dtype=np.float32)
    logstd = np.asarray(logstd, dtype=np.float32)
    B = np.asarray(B, dtype=np.float32)
    eps = np.asarray(eps, dtype=np.float32)
    b = int(batch_size)
    assert B.shape == (1, M, M) and eps.shape == (b, M, 1) and b == BATCH

    in_maps, mu_rep, logstd_rep = _prep_inputs(mu, logstd, B, eps)

    nc = _get_nc()
    kw = {}
    if trace:
        kw = dict(trace=True, trace_cores=list(range(NCORES)))
        if trace_kwargs:
            kw.update(trace_kwargs)
    res = bass_utils.run_bass_kernel_spmd(
        nc, in_maps, core_ids=list(range(NCORES)), **kw
    )

    samples_bm = np.concatenate(
        [res.results[c]["out"] for c in range(NCORES)], axis=1
    )  # (BATCH, M)
    samples = samples_bm.reshape(b, NS, Z)
    mu_out = np.broadcast_to(mu_rep[None, :], (b, M)).reshape(b, NS, Z).copy()
    logvar = (
        np.broadcast_to(2.0 * logstd_rep[None, :], (b, M)).reshape(b, NS, Z).copy()
    )
    return (mu_out, logvar, samples), res


def kernel(mu, logstd, B, eps, batch_size):
    outs, _ = _run(mu, logstd, B, eps, batch_size, trace=False)
    return outs


# revision 16
# speedup vs baseline: 1.1176x; 1.0173x over previous
"""TRN2 Bass kernel for nn_COV_75359496176097.

reference():
    B2 = B[0]                               # (8192, 8192)
    rn = sqrt(1 / sum(B2*B2, axis=1))       # row norms
    A  = rn * B2 * exp(tile(logstd, 64))[:, None]
    samples = tile(mu,64) + einsum('mk,bk->bm', A, eps[:,:,0])
    returns (mu_out, logvar, samples), each (128, 64, 128)

Strategy: shard B by rows across 8 cores (1024 rows each, no collectives).
Each core computes out[b, r] = sum_k eps[k, b] * B[r, k] on the PE
(eps k-tile stationary fp32r, B^T k-tile moving fp32r, PSUM-accumulated
over 64 k-tiles). Row norms: ACT squares each B^T tile (fp32), DVE
accumulates the squares elementwise across k-tiles (ping-pong pair of
accumulators to avoid back-to-back pipeline hazards), and a single pair
of all-ones fp32 matmuls in the tail does the 128-partition reduction —
broadcast across all output partitions for free. Epilogue applies
out = acc * sqrt(1/sumsq) * exp(logstd_rep) + mu_rep and DMAs out.

Raw Bass (not Tile): hardware allows at most ONE semaphore wait per
instruction, and this dataflow (each DMA'd tile consumed by PE and ACT)
needs transitive cross-engine reasoning Tile doesn't do. Manual scheme:
per-slot DMA-completion semaphores; DVE's accumulate for tile t waits on
both PE (via a nop) and ACT, so "DVE retired tile t" implies every
consumer of slot t is done; the DMA issuer throttles on that single DVE
semaphore.

Each k-tile's B^T slice and eps^T slice are packed side by side in one
host-prepared tensor so a k-tile needs exactly one DMA.
"""

import sys
from contextlib import ExitStack

if "/opt/trn_rl_repo" not in sys.path:
    sys.path.insert(0, "/opt/trn_rl_repo")

import numpy as np

import concourse.bacc as bacc
import concourse.mybir as mybir
from concourse import bass_utils
from concourse.dve_ops import RECIPROCAL_APPROX_NR

Z = 128
NS = 64
M = Z * NS          # 8192
BATCH = 128
NCORES = 8
RPC = M // NCORES   # 1024 rows of B per core
KT = M // 128       # 64 k-tiles
W = RPC + BATCH     # 1152 packed row width
NB = 8              # B-tile SBUF slots (DMA prefetch depth)

f32 = mybir.dt.float32
f32r = mybir.dt.float32r
bf16 = mybir.dt.bfloat16

_nc_cache = {}


def _build():
    nc = bacc.Bacc("TRN2", debug=False)

    bte_d = nc.dram_tensor("bte", (M, W), f32r, kind="ExternalInput")
    els_d = nc.dram_tensor("els", (BATCH, RPC), f32, kind="ExternalInput")
    mu_d = nc.dram_tensor("mu", (BATCH, RPC), f32, kind="ExternalInput")
    out_d = nc.dram_tensor("out", (BATCH, RPC), f32, kind="ExternalOutput")

    with ExitStack() as ctx:
        e = ctx.enter_context
        slots = [e(nc.sbuf_tensor(f"slot{i}", [128, W], f32r)) for i in range(NB)]
        sq = [e(nc.sbuf_tensor(f"sq{i}", [128, RPC], bf16)) for i in range(NB)]
        ones = e(nc.sbuf_tensor("ones", [128, 128], bf16))
        els_sb = e(nc.sbuf_tensor("els_sb", [128, RPC], f32))
        mu_sb = e(nc.sbuf_tensor("mu_sb", [128, RPC], f32))
        inv_sb = e(nc.sbuf_tensor("inv_sb", [128, RPC], f32))
        rn_sb = e(nc.sbuf_tensor("rn_sb", [128, RPC], f32))
        scale_sb = e(nc.sbuf_tensor("scale_sb", [128, RPC], f32))
        out_sb = e(nc.sbuf_tensor("out_sb", [128, RPC], f32))
        acc = e(nc.psum_tensor([128, RPC], f32))
        nrm = e(nc.psum_tensor([128, RPC], f32))

        s_dma = [e(nc.semaphore(name=f"s_dma{i}")) for i in range(NB)]
        s_cst = e(nc.semaphore(name="s_cst"))
        s_pe = e(nc.semaphore(name="s_pe"))
        s_dve = e(nc.semaphore(name="s_dve"))
        s_act = e(nc.semaphore(name="s_act"))
        s_x = e(nc.semaphore(name="s_x"))
        s_out = e(nc.semaphore(name="s_out"))
        s_od = e(nc.semaphore(name="s_od"))

        block = e(nc.Block())

        @block.sync
        def _(sync):
            for t in range(KT):
                sl = slice(t * 128, (t + 1) * 128)
                if t == NB:
                    # constants only needed by the epilogue; issue after the
                    # first wave of B-tile DMAs so the PE starts sooner
                    sync.dma_start(els_sb[:], els_d.ap()[:, :]).then_inc(
                        s_cst, 16
                    )
                    sync.dma_start(mu_sb[:], mu_d.ap()[:, :]).then_inc(
                        s_cst, 16
                    )
                if t >= NB:
                    # slot free once PE's norm matmul of tile t-NB retired
                    # (transitively implies DVE's square is done too)
                    sync.wait_ge(s_pe, t - NB + 1)
                sync.dma_start(
                    slots[t % NB][:], bte_d.ap()[sl, :]
                ).then_inc(s_dma[t % NB], 16)
            sync.wait_ge(s_out, 1)
            sync.dma_start(out_d.ap()[:, :], out_sb[:]).then_inc(s_od, 16)
            sync.wait_ge(s_od, 16)
            sync.nop()

        @block.tensor
        def _(tensor):
            for t in range(KT):
                st, sp = t == 0, t == KT - 1
                s = t % NB
                tensor.wait_ge(s_dma[s], 16 * (t // NB + 1))
                eps_v = slots[s][:, RPC:W]
                for h in range(RPC // 512):
                    hs = slice(h * 512, (h + 1) * 512)
                    nc.tensor.matmul(
                        acc[:, hs], eps_v, slots[s][:, hs], start=st, stop=sp
                    )
                # bf16 norm matmuls double as LDW shadow for the fp32r pair
                tensor.wait_ge(s_dve, t + 1)
                for h in range(RPC // 512):
                    hs = slice(h * 512, (h + 1) * 512)
                    ins = nc.tensor.matmul(
                        nrm[:, hs], ones[:], sq[s][:, hs], start=st, stop=sp
                    )
                ins.then_inc(s_pe, 1)

        @block.scalar
        def _(scalar):
            scalar.wait_ge(s_x, 2)
            nc.scalar.sqrt(rn_sb[:], inv_sb[:]).then_inc(s_x, 1)

        @block.vector
        def _(vector):
            nc.vector.memset(ones[:], 1.0)
            for t in range(KT):
                s = t % NB
                # the slot DMA only fired after PE retired tile t-NB, so the
                # sq[s] anti-dependency (PE read of square t-NB) is implied
                vector.wait_ge(s_dma[s], 16 * (t // NB + 1))
                btf = slots[s][:, 0:RPC].bitcast(f32)
                nc.vector.tensor_mul(sq[s][:], btf, btf).then_inc(s_dve, 1)
            # epilogue: out = acc * sqrt(1/nrm) * els + mu
            vector.wait_ge(s_pe, KT)
            nc.vector.reciprocal_approx_fast(out=scale_sb[:], in_=nrm[:])
            vector.drain()
            nc.vector._custom_dve(
                RECIPROCAL_APPROX_NR,
                out=inv_sb[:],
                in0=nrm[:],
                in1=scale_sb[:],
                s0=2.0,
            ).then_inc(s_x, 2)
            vector.wait_ge(s_cst, 32)
            vector.drain()
            vector.wait_ge(s_x, 3)
            nc.vector.tensor_mul(scale_sb[:], rn_sb[:], els_sb[:])
            vector.drain()
            nc.vector.tensor_mul(out_sb[:], acc[:], scale_sb[:])
            vector.drain()
            nc.vector.tensor_add(out_sb[:], out_sb[:], mu_sb[:]).then_inc(
                s_out, 1
            )

    nc.compile()
    return nc


def _get_nc():
    if "nc" not in _nc_cache:
        _nc_cache["nc"] = _build()
    return _nc_cache["nc"]


def _prep_inputs(mu, logstd, B, eps):
    B2 = B[0]
    epst = np.ascontiguousarray(eps[:, :, 0].T)        # (M, BATCH)
    mu_rep = np.tile(mu[0], NS)                        # (M,)
    logstd_rep = np.tile(logstd, NS)                   # (M,)
    els_rep = np.exp(logstd_rep).astype(np.float32)    # (M,)

    in_maps = []
    for c in range(NCORES):
        rows = slice(c * RPC, (c + 1) * RPC)
        bte = np.empty((M, W), dtype=np.float32)
        bte[:, 0:RPC] = B2[rows, :].T
        bte[:, RPC:W] = epst
        in_maps.append(
            {
                "bte": bte,
                "els": np.ascontiguousarray(
                    np.broadcast_to(els_rep[rows][None, :], (BATCH, RPC))
                ),
                "mu": np.ascontiguousarray(
                    np.broadcast_to(mu_rep[rows][None, :], (BATCH, RPC))
                ),
            }
        )
    return in_maps, mu_rep, logstd_rep


def _run(mu, logstd, B, eps, batch_size, trace=False, trace_kwargs=None):
    mu = np.asarray(mu, dtype=np.float32)
    logstd = np.asarray(logstd, dtype=np.float32)
    B = np.asarray(B, dtype=np.float32)
    eps = np.asarray(eps, dtype=np.float32)
    b = int(batch_size)
    assert B.shape == (1, M, M) and eps.shape == (b, M, 1) and b == BATCH

    in_maps, mu_rep, logstd_rep = _prep_inputs(mu, logstd, B, eps)

    nc = _get_nc()
    kw = {}
    if trace:
        kw = dict(trace=True, trace_cores=list(range(NCORES)))
        if trace_kwargs:
            kw.update(trace_kwargs)
    res = bass_utils.run_bass_kernel_spmd(
        nc, in_maps, core_ids=list(range(NCORES)), **kw
    )

    samples_bm = np.concatenate(
        [res.results[c]["out"] for c in range(NCORES)], axis=1
    )  # (BATCH, M)
    samples = samples_bm.reshape(b, NS, Z)
    mu_out = np.broadcast_to(mu_rep[None, :], (b, M)).reshape(b, NS, Z).copy()
    logvar = (
        np.broadcast_to(2.0 * logstd_rep[None, :], (b, M)).reshape(b, NS, Z).copy()
    )
    return (mu_out, logvar, samples), res


def kernel(mu, logstd, B, eps, batch_size):
    outs, _ = _run(mu, logstd, B, eps, batch_size, trace=False)
    return outs
